# revision 20
# baseline (speedup 1.0000x reference)
"""Trainium2 Bass kernel for the DeformableCurrents loss.

Energy e = e_ss - 2*e_st + e_tt where e_xy = sum_ij K(c_i, c_j) * <n_i, n_j>
with the Cauchy kernel K = 1/(1 + |ci - cj|^2).

Strategy (8-core SPMD, identical instruction stream per core):
  - Work units: [128 j] x [512 i] blocks of the pairwise kernel matrices,
    grouped 4 blocks to a "pg" (one i-chunk x 512 j's).
  - Each core runs 4 uniform SEGMENTS of [16, 17, 16, 17] pgs. A segment
    keeps ONE i-chunk fixed: jobs S_c (i = source chunk c: ss triangle rows
    + c st superblocks) and T_c (i = target chunk c: tt rows + c+1 st
    superblocks). The st superblock (a, b) is computed with source-chunk
    orientation iff a > b, making every S_c exactly 16 pgs and every T_c
    exactly 17 -- the same segment schedule on all 8 cores (SPMD-safe).
  - P-matmul (K=5 fp16): P[j, i] = 1 + |y_j - x_i|^2 via augmented
    features. The 4 blocks of a pg are ROW-TILED at tile_position (32r, 0)
    (features staged at partition strips 32r..32r+4) so the 4 matmuls run
    concurrently in different row groups of the PE array and their
    LDWEIGHTS (FWL-eligible for fp16) overlap in-flight matmuls.
  - Reciprocal split across engines: DVE custom fast-reciprocal for the two
    single-bank B tiles, ScalarE ACTIVATE(Reciprocal) for the 2-bank A
    tile; both write one bf16 [128, 2048] pid tile in SBUF.
    PSUM budget (8 banks): psA 2x2 + psB 3x1 + sS 1 -- the B tiles are
    single-bank with bufs=3 so the PE->DVE->PE reuse chain has 1.5 pgs of
    slack to hide the ~0.5us cross-engine semaphore latency.
  - S-matmul (K=128, M=3, bf16): 4 col-tiled matmuls (tile_position
    (0, 32q)) accumulate S[d, i] in ONE PSUM bank across the whole
    segment, emitted 2 pgs behind the P pipeline for slack. Egress: one
    ACT copy [99, 512] per segment (4 per core).
  - wfeat staged per segment (double-buffered, prefetched one segment
    ahead); rhsf/wnrm staged once. Host computes the final dot
    sum_{d,i} S[d,i] * n[d,i] per segment.
"""

import numpy as np

V, N, M = 4096, 8192, 8192
CHUNK = 512
BLOCK = 128
NCORES = 8
SEGS = [16, 17, 16, 17]
PGS_PER_CORE = 66
_ACTIVE_PGS = None  # test hook: if set, only this many pgs are emitted
_REPEAT = 1         # test hook: emit the whole pg loop this many times
_LOOP_R = None      # test hook: wrap the body in a device-side For_i loop
_STAGE_MODE = "full"  # test hook: full | noegress | nomms | mmp

_CACHED_NC = None

# segment index / boundary tables (same for every core)
_SEG_OF = []
_SEG_FIRST = []
_SEG_LAST = []
for _s, _l in enumerate(SEGS):
    _SEG_OF += [_s] * _l
    _SEG_FIRST += [True] + [False] * (_l - 1)
    _SEG_LAST += [False] * (_l - 1) + [True]


# ---------------------------------------------------------------- planning
def _plan():
    """Per-core list of 4 segments; seg = (kind, chunk, quads),
    quad = (jside, blocks[4], w)."""
    def S_job(c):
        quads = [("src", list(range(4 * c, 4 * c + 4)), 1.0)]
        quads += [("src", list(range(k, k + 4)), 2.0)
                  for k in range(4 * c + 4, 64, 4)]
        quads += [("tar", list(range(4 * b, 4 * b + 4)), -2.0)
                  for b in range(c)]
        return ("S", c, quads)

    def T_job(c):
        quads = [("tar", list(range(4 * c, 4 * c + 4)), 1.0)]
        quads += [("tar", list(range(k, k + 4)), 2.0)
                  for k in range(4 * c + 4, 64, 4)]
        quads += [("src", list(range(4 * a, 4 * a + 4)), -2.0)
                  for a in range(c + 1)]
        return ("T", c, quads)

    cores = []
    for k in range(NCORES):
        segs = [S_job(2 * k), T_job(2 * k), S_job(2 * k + 1), T_job(2 * k + 1)]
        assert [len(s[2]) for s in segs] == SEGS
        cores.append(segs)
    return cores


# ---------------------------------------------------------------- bass build
def _build_nc():
    global _CACHED_NC
    if _CACHED_NC is not None:
        return _CACHED_NC

    from contextlib import ExitStack, nullcontext

    import concourse.bass as bass
    import concourse.tile as tile
    from concourse import bacc, mybir
    from concourse.dve_ops import RECIP_APPROX_FAST_CONSTS, RECIPROCAL_APPROX_FAST

    F32 = mybir.dt.float32
    F32R = mybir.dt.float32r
    F16 = mybir.dt.float16
    BF16 = mybir.dt.bfloat16
    AF = mybir.ActivationFunctionType

    nc = bacc.Bacc("TRN2", target_bir_lowering=False, debug=False,
                   num_devices=NCORES)

    # Pin Reciprocal/Copy to the one table set containing both so the
    # table-load fixpoint emits a single LoadActFuncSet.
    from concourse.hw_specs import get_activation_tables
    _tabs = get_activation_tables(nc.m.arch)
    _pinned = {AF.Reciprocal, AF.Copy}
    if "reciprocal_and_small" in _tabs:
        for _name, _fns in _tabs.items():
            if _name != "reciprocal_and_small":
                _fns -= _pinned

    def act_recip(out, in_):
        # nc.scalar.activation refuses AF.Reciprocal (generic-accuracy
        # guard); the spline's error is far below this kernel's bf16
        # output rounding, so emit the ACTIVATE directly.
        sc = nc.scalar
        ins = [sc.lower_ap(in_)]
        for immv in (0.0, 1.0, 0.0):  # bias, scale, alpha
            ins.append(mybir.ImmediateValue(dtype=mybir.dt.float32, value=immv))
        return sc.add_instruction(
            mybir.InstActivation(
                name=nc.get_next_instruction_name(),
                func=AF.Reciprocal,
                ins=ins,
                outs=[sc.lower_ap(out)],
            )
        )

    # Feature slabs, row-tiling layout: dram row 5r+k = feature k of the
    # pg's j-block r; staged at SBUF partition strip 32r..32r+4. rhsf is
    # the i-chunk feature block of each segment, host-replicated per strip.
    wfeat_d = nc.dram_tensor("wfeat", [20, PGS_PER_CORE, 128], F16,
                             kind="ExternalInput").ap()
    rhsf_d = nc.dram_tensor("rhsf", [20, len(SEGS), 512], F16,
                            kind="ExternalInput").ap()
    wnrm_d = nc.dram_tensor("wnrm", [128, PGS_PER_CORE, 12], BF16,
                            kind="ExternalInput").ap()
    # S results: segment s -> cols 512s.., rows 32q+d (strip q, dim d)
    sout_d = nc.dram_tensor("sout", [99, len(SEGS) * 512], F32,
                            kind="ExternalOutput").ap()

    rc = RECIP_APPROX_FAST_CONSTS

    with tile.TileContext(nc) as tc, ExitStack() as ctx:
        const = ctx.enter_context(tc.tile_pool(name="const", bufs=1))
        stage = ctx.enter_context(tc.tile_pool(name="stage", bufs=2))
        piv = ctx.enter_context(tc.tile_pool(name="piv", bufs=3))
        outp = ctx.enter_context(tc.tile_pool(name="outp", bufs=1))
        # PSUM bank budget (8): psA 2x2 (ACT input, double-buffered) +
        # psB 3x1 (DVE input, single-bank tiles so the PE->DVE->PE reuse
        # chain has 1.5 pgs of slack to hide semaphore latency) + sS 1
        psA = ctx.enter_context(
            tc.tile_pool(name="psA", bufs=2, space=bass.MemorySpace.PSUM))
        psB = ctx.enter_context(
            tc.tile_pool(name="psB", bufs=3, space=bass.MemorySpace.PSUM))
        sP = ctx.enter_context(
            tc.tile_pool(name="sP", bufs=1, space=bass.MemorySpace.PSUM))

        sout = outp.tile([99, len(SEGS) * 512], F32, tag="sout")
        mode = _STAGE_MODE
        if mode != "full":
            sink = outp.tile([1, 64], F32, tag="sink")
        else:
            sink = None

        n_active = _ACTIVE_PGS if _ACTIVE_PGS is not None else PGS_PER_CORE

        loop_cm = (tc.For_i(0, _LOOP_R, 1) if _LOOP_R else nullcontext())
        with loop_cm:
          for p0 in range(n_active * _REPEAT):
            p = p0 % n_active
            if p == 0:
                seg_tiles = {}
                wf_tiles = {}
                pend = []
                # rhsf/wnrm staged whole-kernel; wfeat per segment (below)
                rhsf_t = const.tile([101, len(SEGS), 512], F16, tag="rhsf")
                wnrm_t = const.tile([128, PGS_PER_CORE, 12], BF16, tag="wnrm")
                if p0 == 0 or _LOOP_R:
                    for r in range(4):
                        nc.gpsimd.dma_start(rhsf_t[32 * r : 32 * r + 5, :, :],
                                            rhsf_d[5 * r : 5 * r + 5, :, :])
                    nc.gpsimd.dma_start(wnrm_t[:], wnrm_d[:])

            def stage_wfeat(sg):
                # double-buffered per-segment feature staging: segment sg+1
                # prefetches while sg computes
                p0s = sum(SEGS[:sg])
                ln = min(SEGS[sg], max(0, n_active - p0s))
                if ln <= 0:
                    return
                wt = stage.tile([101, max(SEGS), 128], F16, tag="wseg")
                for r in range(4):
                    nc.sync.dma_start(
                        wt[32 * r : 32 * r + 5, 0:ln, :],
                        wfeat_d[5 * r : 5 * r + 5, p0s : p0s + ln, :])
                wf_tiles[sg] = wt

            seg = _SEG_OF[p]
            if p == 0:
                stage_wfeat(0)
                stage_wfeat(1)
            elif _SEG_FIRST[p] and seg + 1 < len(SEGS):
                stage_wfeat(seg + 1)
            if _SEG_FIRST[p] and mode not in ("mmp", "nomms"):
                sS_new = sP.tile([99, 512], F32, tag="sS")
                # define the unused partitions so the [99, 512] egress reads
                # initialized memory owned by this tile
                nc.vector.memset(sS_new[:], 0.0)
                seg_tiles[seg] = sS_new
            wnrm_s = wnrm_t[:, p, :]
            wfeat_t = wf_tiles[seg]
            poff = p - sum(SEGS[:seg])

            # ---- P matmuls, row-tiled: strip r computes block r
            # blocks 0,1 -> psB single-bank tiles; blocks 2,3 -> psA halves
            ps_b0 = psB.tile([128, 512], F32, tag="psb")
            ps_b1 = psB.tile([128, 512], F32, tag="psb")
            ps_a = psA.tile([128, 1024], F32, tag="psa")
            for r in range(4):
                out = (ps_b0 if r == 0 else ps_b1 if r == 1
                       else ps_a[:, 512 * (r - 2) : 512 * (r - 1)])
                nc.tensor.matmul(out,
                                 wfeat_t[32 * r : 32 * r + 5, poff, :],
                                 rhsf_t[32 * r : 32 * r + 5, seg, :],
                                 start=True, stop=True,
                                 tile_position=(32 * r, 0))

            if mode == "mmp":
                nc.vector.tensor_copy(sink[:, 4:8], ps_b0[0:1, 0:4])
                nc.vector.tensor_copy(sink[:, 8:12], ps_a[0:1, 0:4])
                continue

            # ---- reciprocals: DVE takes the two B banks, ACT takes tile A
            pid = piv.tile([128, 2048], BF16, tag="pid")
            nc.vector._custom_dve(RECIPROCAL_APPROX_FAST, out=pid[:, 0:512],
                                  in0=ps_b0[:], s0=rc["s0"], s1=rc["s1"],
                                  imm2=rc["imm2"])
            nc.vector._custom_dve(RECIPROCAL_APPROX_FAST, out=pid[:, 512:1024],
                                  in0=ps_b1[:], s0=rc["s0"], s1=rc["s1"],
                                  imm2=rc["imm2"])
            act_recip(pid[:, 1024:2048], ps_a[:])

            if mode == "nomms":
                nc.vector.tensor_copy(sink[:, 20:24], pid[0:1, 0:4])
                nc.vector.tensor_copy(sink[:, 24:28], pid[0:1, 1024:1028])
                continue

            # ---- previous pg's S matmuls follow this pg's P matmuls in the
            # PE stream (PE never waits on this pg's reciprocals)
            def emit_smms(q_prev):
                pp, pid_p, wnrm_p = q_prev
                sg = _SEG_OF[pp]
                sS = seg_tiles[sg]
                first = _SEG_FIRST[pp]
                last = _SEG_LAST[pp] or pp == n_active - 1
                for q in range(4):
                    nc.tensor.matmul(sS[32 * q : 32 * q + 3, :],
                                     wnrm_p[:, 3 * q : 3 * (q + 1)],
                                     pid_p[:, 512 * q : 512 * (q + 1)],
                                     start=first, stop=last,
                                     tile_position=(0, 32 * q))
                if last:
                    if mode == "noegress":
                        nc.vector.tensor_copy(sink[:, 32:36], sS[0:1, 0:4])
                    else:
                        nc.scalar.activation(
                            sout[:, 512 * sg : 512 * (sg + 1)], sS[:], AF.Copy)

            pend.append((p, pid, wnrm_s))
            if len(pend) > 2:
                emit_smms(pend.pop(0))

          # pipeline flush (inside the optional timing loop)
          for q_prev in pend:
              emit_smms(q_prev)
          pend = []

        if mode == "full":
            nc.sync.dma_start(sout_d[:], sout[:])
        else:
            nc.sync.dma_start(sout_d[0:1, 0:64], sink[:])

    nc.compile()
    _CACHED_NC = nc
    return nc


# ---------------------------------------------------------------- host side
def _feats(pts):
    """pts [n,3] f32 -> featL [5,n] (lhsT side), featR [5,n] (rhs side)."""
    x, y, z = pts[:, 0], pts[:, 1], pts[:, 2]
    n2 = x * x + y * y + z * z
    one = np.ones_like(n2)
    featL = np.stack([x, y, z, n2, one]).astype(np.float32)
    featR = np.stack([-2 * x, -2 * y, -2 * z, one, n2 + 1.0]).astype(np.float32)
    return featL, featR


def kernel(src_vertices, tar_normals, tar_centers, src_indices):
    import ml_dtypes
    from concourse.bass_utils import run_bass_kernel_spmd

    src_vertices = np.asarray(src_vertices, dtype=np.float32)
    tar_normals = np.asarray(tar_normals, dtype=np.float32)
    tar_centers = np.asarray(tar_centers, dtype=np.float32)
    idx = np.asarray(src_indices).astype(np.int64)

    # triangle gather: normals and centers of source triangles
    tris = src_vertices[idx]                      # [N, 3, 3]
    a, b, c = tris[:, 0, :], tris[:, 1, :], tris[:, 2, :]
    normals = 0.5 * np.cross(a - b, c - b).astype(np.float32)   # [N,3]
    centers = (tris.sum(axis=1) / 3.0).astype(np.float32)       # [N,3]

    sfL, sfR = _feats(centers)
    tfL, tfR = _feats(tar_centers)

    featL = {"src": sfL, "tar": tfL}            # j side [5, n]
    featR = {"S": sfR, "T": tfR}                # i side [5, n]
    nrmJ = {"src": normals, "tar": tar_normals}  # [n, 3] j side
    fnI = {"S": normals, "T": tar_normals}       # [n, 3] i side (host dot)

    cores = _plan()
    in_maps = []
    fin = []  # per core: list of segment fn [512, 3] f64
    for core in range(NCORES):
        segs = cores[core]
        wfeat = np.empty((20, PGS_PER_CORE, 128), np.float32)
        rhsf = np.empty((20, len(SEGS), 512), np.float32)
        wnrm = np.empty((PGS_PER_CORE, 128, 12), np.float32)
        fns = []
        p = 0
        for si, (kind, cc, quads) in enumerate(segs):
            fr = featR[kind][:, CHUNK * cc : CHUNK * (cc + 1)]
            for r in range(4):
                rhsf[5 * r : 5 * r + 5, si, :] = fr
            fns.append(fnI[kind][CHUNK * cc : CHUNK * (cc + 1), :]
                       .astype(np.float64))
            for (jside, blocks, w) in quads:
                for q, blk in enumerate(blocks):
                    wfeat[5 * q : 5 * q + 5, p, :] = (
                        featL[jside][:, BLOCK * blk : BLOCK * (blk + 1)])
                    wnrm[p, :, 3 * q : 3 * (q + 1)] = (
                        w * nrmJ[jside][BLOCK * blk : BLOCK * (blk + 1), :])
                p += 1
        assert p == PGS_PER_CORE
        in_maps.append({
            "wfeat": np.ascontiguousarray(wfeat).astype(np.float16),
            "rhsf": np.ascontiguousarray(rhsf).astype(np.float16),
            "wnrm": np.ascontiguousarray(
                wnrm.transpose(1, 0, 2)).astype(ml_dtypes.bfloat16),
        })
        fin.append(fns)

    nc = _build_nc()
    results = run_bass_kernel_spmd(nc, in_maps, list(range(NCORES))).results

    e = 0.0
    for core in range(NCORES):
        sout = np.asarray(results[core]["sout"], dtype=np.float64)  # [99, 4*512]
        for si in range(len(SEGS)):
            blkS = sout[:, 512 * si : 512 * (si + 1)]
            S3 = sum(blkS[32 * q : 32 * q + 3, :] for q in range(4))  # [3,512]
            e += float((S3.T * fin[core][si]).sum())
    return np.float32(e)
